# revision 4
# baseline (speedup 1.0000x reference)
"""NetVLAD pooling kernel v2 for Trainium2 (8 NeuronCores, data-parallel over B).

Per core: 32 tokens, r = [N=2048, C=64] each; logits = r @ W.T + b;
a = softmax(logits); v = a.T @ r - sum(a).T * centroids   -> [K=32, C=64].

Design (vs the two-GEMM baseline, measured on HW):
  - GEMM1 runs "transposed" with W STATIONARY: the baseline made each rT chunk
    the stationary operand, paying a 128-col LDWEIGHTS (~107ns) per chunk
    (~55us/core of PE weight-load). Here 4 copies of W.T sit in the PE at
    tile_position (64*(j%2), 32*j) (row-pair x 4 col-strips, one per token of
    a 4-token batch) and r streams as the MOVING operand in N=512 quarters.
    Issuing the 2 same-weight quarters back-to-back pairs the streams in the
    array: measured ~108ns per N=512 matmul -> GEMM1 ~14us/core.
    Output: logitsT psum [128 = 4 tok x 32 k, 512 n].
  - b folds into the exp via ACTIVATE's per-partition bias (k is the partition
    dim here), deleting the baseline's beta-multiply pass on VectorE.
  - a-tiles for GEMM2 need [n, k] layout: PE transpose-mode on the exp'd bf16
    tiles ([128,128] blocks, ~108-200ns each, output psum-bf16; measured exact)
    -> [n-part, (cq, tok, k)]. Softmax denominator = VectorE segmented reduce
    on the psum-bf16 tile; 1/s on GpSimd (ALU divide, bf16 out); the normalize
    multiply doubles as the psum->SBUF evacuation (all-bf16 for DVE rate).
  - GEMM2 = baseline scheme: per (chunk, token) a-tile [128, 32] stationary,
    rhs = RN [128 n, 65] with a trailing -1 column so psum col 64 = -sum(a);
    4 tokens col-tiled into one psum bank; measured ~33ns/MM -> ~17us/core.
  - Epilogue: v = c4 * pv[:, 64] + pv[:, :64] (one scalar_tensor_tensor).
  - GEMM2 lags the softmax chain by one batch; loads prefetch one batch ahead.
"""

import os
import sys

import numpy as np

sys.path.insert(0, "/opt/trn_rl_repo")

import ml_dtypes  # noqa: E402

import concourse.bass as bass  # noqa: E402
import concourse.tile as tile  # noqa: E402
from concourse import mybir  # noqa: E402
from concourse.bass_utils import run_bass_kernel_spmd  # noqa: E402

B, T, N, C, K = 8, 32, 2048, 64, 32
NCORES = 8
TOK = (B * T) // NCORES     # 32
NBATCH = TOK // 4           # 8
NPAIR = TOK // 2            # 16
NCH = N // 128              # 16

BF16 = mybir.dt.bfloat16
F32 = mybir.dt.float32

_CACHE = {}

_NO_SPLIT_TYPES = ("InstEventSemaphore",)


def _split_excess_waits(nc):
    """walrus' setupSyncWait allows a single semaphore wait on several
    instruction structs; hoist extras onto standalone InstEventSemaphore."""
    for f in nc.m.functions:
        for blk in f.blocks:
            out = []
            changed = False
            for inst in blk.instructions:
                si = getattr(inst, "sync_info", None)
                if (
                    si is not None
                    and si.on_wait
                    and len(si.on_wait) > 1
                    and type(inst).__name__ not in _NO_SPLIT_TYPES
                ):
                    for idx, w in enumerate(si.on_wait[:-1]):
                        out.append(
                            mybir.InstEventSemaphore(
                                name=f"{inst.name}_xw{idx}",
                                engine=inst.engine,
                                sync_info=mybir.SyncInfo(on_wait=[w], on_update=[]),
                            )
                        )
                    inst.sync_info = mybir.SyncInfo(
                        on_wait=[si.on_wait[-1]], on_update=si.on_update
                    )
                    changed = True
                out.append(inst)
            if changed:
                try:
                    blk.instructions[:] = out
                except TypeError:
                    blk.instructions = out
    return nc


def _build_nc():
    nc = bass.Bass()
    RT = nc.declare_dram_parameter("RT", [NPAIR, 128, N], BF16, False)
    RN = nc.declare_dram_parameter("RN", [NBATCH, 128, NCH, 4, C + 1], BF16, False)
    W2 = nc.declare_dram_parameter("W2", [128, K], BF16, False)
    B128 = nc.declare_dram_parameter("B128", [128, 1], F32, False)
    IDN = nc.declare_dram_parameter("IDN", [128, 128], BF16, False)
    C4 = nc.declare_dram_parameter("C4", [128, C], F32, False)
    V = nc.declare_dram_parameter("V", [NBATCH, 128, C], F32, True)

    with tile.TileContext(nc) as tc:
        with (
            tc.tile_pool(name="singles", bufs=1) as singles,
            tc.tile_pool(name="rt", bufs=6) as rt_pool,
            tc.tile_pool(name="rn", bufs=3) as rn_pool,
            tc.tile_pool(name="e", bufs=5) as e_pool,
            tc.tile_pool(name="a", bufs=2) as a_pool,
            tc.tile_pool(name="s", bufs=2) as s_pool,
            tc.tile_pool(name="rs", bufs=3) as rs_pool,
            tc.tile_pool(name="o", bufs=2) as o_pool,
            tc.tile_pool(name="pq", bufs=2, space="PSUM") as pq_pool,
            tc.tile_pool(name="pet", bufs=3, space="PSUM") as pet_pool,
            tc.tile_pool(name="pv", bufs=1, space="PSUM") as pv_pool,
        ):
            # W first: it gates the first matmul (rt halves issue right after,
            # in the main loop preamble below)
            w_sb = singles.tile([128, K], BF16)
            nc.sync.dma_start(out=w_sb[:], in_=W2[:])
            b_sb = singles.tile([128, 1], F32)
            idn_sb = singles.tile([128, 128], BF16)
            c4_sb = singles.tile([128, C], F32)

            def load_singles_rest():
                nc.sync.dma_start(out=b_sb[:], in_=B128[:])
                nc.sync.dma_start(out=idn_sb[:], in_=IDN[:])
                nc.sync.dma_start(out=c4_sb[:], in_=C4[:])

            # force the Exp ACT table load during the DMA-fill window instead
            # of right before the first real exp
            warm = singles.tile([128, 1], F32)
            nc.vector.memset(warm[:], 0.0)
            nc.scalar.activation(
                warm[:], warm[:], mybir.ActivationFunctionType.Exp
            )

            rt_sb = [None] * NPAIR
            rn_sb = [None] * NBATCH
            a_sb = [None] * NBATCH   # per batch: 4 quarter tiles [128, 4, 4, K]
            pv = [None] * NBATCH

            def load_pair(p, split=False):
                rt_sb[p] = rt_pool.tile([128, N], BF16, name="rt_t", tag="rt_t")
                if split:
                    # two half-loads so the first G1 quarters start sooner
                    nc.sync.dma_start(
                        out=rt_sb[p][:, 0 : N // 2], in_=RT[p][:, 0 : N // 2]
                    )
                    nc.sync.dma_start(
                        out=rt_sb[p][:, N // 2 : N], in_=RT[p][:, N // 2 : N]
                    )
                else:
                    nc.sync.dma_start(out=rt_sb[p][:], in_=RT[p])

            def load_rn(bt):
                rn_sb[bt] = rn_pool.tile(
                    [128, NCH, 4, C + 1], BF16, name="rn_t", tag="rn_t"
                )
                nc.sync.dma_start(out=rn_sb[bt][:], in_=RN[bt])

            def g1(bt):
                """16 matmuls -> two [128, 2, 512] psum tiles; exp -> e quarters."""
                e_q = []
                for Qh in range(2):
                    ps2 = pq_pool.tile([128, 2, 512], F32, name="ps_t", tag="ps_t")
                    for j in range(4):
                        pr = 2 * bt + j // 2
                        q = j % 2
                        for Qq in range(2):
                            Q = 2 * Qh + Qq
                            nc.tensor.matmul(
                                ps2[32 * j : 32 * j + 32, Qq, :],
                                w_sb[64 * q : 64 * q + 64, :],
                                rt_sb[pr][64 * q : 64 * q + 64, 512 * Q : 512 * Q + 512],
                                start=True,
                                stop=True,
                                skip_group_check=True,
                                tile_position=(64 * q, 32 * j),
                            )
                    for Qq in range(2):
                        e = e_pool.tile([128, 512], BF16, name="e_t", tag="e_t")
                        nc.scalar.activation(
                            e[:],
                            ps2[:, Qq, :],
                            mybir.ActivationFunctionType.Exp,
                            bias=b_sb[:],
                        )
                        e_q.append(e)
                return e_q

            def softmax_quarter(bt, Q, e, s_b):
                """Transpose e-quarter (PE), reduce over k, 1/s, normalize."""
                et = pet_pool.tile([128, 4, 4, K], BF16, name="et_t", tag="et_t")
                for cq in range(4):
                    nc.tensor.transpose(
                        et[:, cq, :, :],
                        e[:, 128 * cq : 128 * cq + 128],
                        idn_sb[:],
                    )
                nc.vector.tensor_reduce(
                    s_b[:, Q, :, :],
                    et[:],
                    axis=mybir.AxisListType.X,
                    op=mybir.AluOpType.add,
                )
                rs_q = rs_pool.tile([128, 4, 4], F32, name="rs_t", tag="rs_t")
                nc.vector.reciprocal(rs_q[:], s_b[:, Q, :, :])
                aq = a_pool.tile([128, 4, 4, K], BF16, name=f"a{Q}_t", tag=f"a{Q}_t")
                nc.vector.tensor_mul(
                    aq[:], et[:], rs_q[:].unsqueeze(3).broadcast_to((128, 4, 4, K))
                )
                return aq

            def g2_quarter(bt, Q):
                """16 GEMM2 matmuls for chunks 4Q..4Q+3 (accumulating into pv)."""
                if Q == 0:
                    pv[bt] = pv_pool.tile([128, C + 1], F32, name="pv_t", tag="pv_t")
                pvt = pv[bt]
                rn = rn_sb[bt]
                aq = a_sb[bt][Q]
                for cq in range(4):
                    ch = 4 * Q + cq
                    for j in range(4):
                        nc.tensor.matmul(
                            pvt[32 * j : 32 * j + 32, :],
                            aq[:, cq, j, :],
                            rn[:, ch, j, :],
                            start=(ch == 0),
                            stop=(ch == NCH - 1),
                            skip_group_check=True,
                            tile_position=(0, 32 * j),
                        )

            def epilogue(bt):
                o = o_pool.tile([128, C], F32, name="o_t", tag="o_t")
                nc.vector.scalar_tensor_tensor(
                    o[:],
                    c4_sb[:],
                    pv[bt][:, C : C + 1],
                    pv[bt][:, 0:C],
                    op0=mybir.AluOpType.mult,
                    op1=mybir.AluOpType.add,
                )
                nc.sync.dma_start(out=V[bt], in_=o[:])
                pv[bt] = None
                rn_sb[bt] = None
                a_sb[bt] = None

            # first halves of both pairs first: they gate the first MM group
            for p in (0, 1):
                rt_sb[p] = rt_pool.tile([128, N], BF16, name="rt_t", tag="rt_t")
            for half in (0, 1):
                for p in (0, 1):
                    nc.sync.dma_start(
                        out=rt_sb[p][:, half * (N // 2) : (half + 1) * (N // 2)],
                        in_=RT[p][:, half * (N // 2) : (half + 1) * (N // 2)],
                    )
            load_singles_rest()
            for bt in range(NBATCH + 1):
                if bt < NBATCH:
                    if 2 * bt + 3 < NPAIR:
                        load_pair(2 * bt + 2)
                        load_pair(2 * bt + 3)
                    load_rn(bt)
                    e_q = g1(bt)
                    # interleave transpose bursts (HAM-idle for the PE) with
                    # GEMM2 matmul groups of the previous batch to keep the
                    # PE clock gate warm; reciprocals batched per quarter-pair
                    s_b = s_pool.tile([128, 4, 4, 4], F32, name="s_t", tag="s_t")
                    quarters = []
                    a_sb[bt] = quarters
                    for Q in range(4):
                        quarters.append(softmax_quarter(bt, Q, e_q[Q], s_b))
                        if bt >= 1:
                            g2_quarter(bt - 1, Q)
                        if bt == NBATCH - 1:
                            # last batch: no lag -- run its GEMM2 as each
                            # quarter's a-tiles are ready
                            g2_quarter(bt, Q)
                    rt_sb[2 * bt] = None
                    rt_sb[2 * bt + 1] = None
                if bt >= 1:
                    epilogue(bt - 1)
    return _split_excess_waits(nc)


def kernel(R_seq, W, b, centroids):
    if "nc" not in _CACHE:
        _CACHE["nc"] = _build_nc()
    nc = _CACHE["nc"]
    bf = ml_dtypes.bfloat16

    WT = np.ascontiguousarray(W.astype(np.float32).T)              # [C, K]
    W2_h = np.concatenate([WT, WT], axis=0).astype(bf)             # [128, K]
    B128_h = np.ascontiguousarray(
        np.tile(b.astype(np.float32), 4).reshape(128, 1)
    )
    IDN_h = np.eye(128, dtype=np.float32).astype(bf)
    C4_h = np.ascontiguousarray(np.tile(centroids.astype(np.float32), (4, 1)))

    r_all = R_seq.astype(np.float32).reshape(NCORES, TOK, N, C)
    in_maps = []
    for i in range(NCORES):
        rc = r_all[i]
        r6 = rc.reshape(NPAIR, 2, N, C).transpose(0, 1, 3, 2)      # [p, t, c, n]
        RT_h = np.ascontiguousarray(r6).reshape(NPAIR, 128, N).astype(bf)
        r7 = rc.reshape(NBATCH, 4, NCH, 128, C).transpose(0, 3, 2, 1, 4)
        RN_h = np.concatenate(
            [r7, np.full(r7.shape[:-1] + (1,), -1.0, np.float32)], axis=-1
        ).astype(bf)                                               # [bt,128,ch,j,65]
        in_maps.append(
            {
                "RT": RT_h,
                "RN": np.ascontiguousarray(RN_h),
                "W2": W2_h,
                "B128": B128_h,
                "IDN": IDN_h,
                "C4": C4_h,
            }
        )

    res = run_bass_kernel_spmd(
        nc,
        in_maps,
        list(range(NCORES)),
        trace=bool(int(os.environ.get("NETVLAD_TRACE", "0"))),
    )
    _CACHE["last_results"] = res

    outs = []
    for i in range(NCORES):
        v = np.asarray(res.results[i]["V"], np.float32)   # [NBATCH, 128, C]
        outs.append(v.reshape(TOK, K, C))
    return np.stack(outs, axis=0).reshape(B, T, K, C).astype(np.float32)


if __name__ == "__main__":
    rng = np.random.default_rng(0)
    R = rng.normal(size=(B, T, N, C)).astype(np.float32)
    W_ = rng.normal(size=(K, C)).astype(np.float32) / 8.0
    b_ = (rng.normal(size=(K,)) * 0.01).astype(np.float32)
    cc = rng.normal(size=(K, C)).astype(np.float32)
    out = kernel(R, W_, b_, cc)
    print(out.shape, out.dtype)


# revision 5
# speedup vs baseline: 1.3261x; 1.3261x over previous
"""NetVLAD pooling kernel v2 for Trainium2 (8 NeuronCores, data-parallel over B).

Per core: 32 tokens, r = [N=2048, C=64] each; logits = r @ W.T + b;
a = softmax(logits); v = a.T @ r - sum(a).T * centroids   -> [K=32, C=64].

Design (vs the two-GEMM baseline, measured on HW):
  - GEMM1 runs "transposed" with W STATIONARY: the baseline made each rT chunk
    the stationary operand, paying a 128-col LDWEIGHTS (~107ns) per chunk
    (~55us/core of PE weight-load). Here 4 copies of W.T sit in the PE at
    tile_position (64*(j%2), 32*j) (row-pair x 4 col-strips, one per token of
    a 4-token batch) and r streams as the MOVING operand in N=512 quarters.
    Issuing the 2 same-weight quarters back-to-back pairs the streams in the
    array: measured ~108ns per N=512 matmul -> GEMM1 ~14us/core.
    Output: logitsT psum [128 = 4 tok x 32 k, 512 n].
  - b folds into the exp via ACTIVATE's per-partition bias (k is the partition
    dim here), deleting the baseline's beta-multiply pass on VectorE.
  - a-tiles for GEMM2 need [n, k] layout: PE transpose-mode on the exp'd bf16
    tiles ([128,128] blocks, ~108-200ns each, output psum-bf16; measured exact)
    -> [n-part, (cq, tok, k)]. Softmax denominator = VectorE segmented reduce
    on the psum-bf16 tile; 1/s on GpSimd (ALU divide, bf16 out); the normalize
    multiply doubles as the psum->SBUF evacuation (all-bf16 for DVE rate).
  - GEMM2 = baseline scheme: per (chunk, token) a-tile [128, 32] stationary,
    rhs = RN [128 n, 65] with a trailing -1 column so psum col 64 = -sum(a);
    4 tokens col-tiled into one psum bank; measured ~33ns/MM -> ~17us/core.
  - Epilogue: v = c4 * pv[:, 64] + pv[:, :64] (one scalar_tensor_tensor).
  - GEMM2 lags the softmax chain by one batch; loads prefetch one batch ahead.
"""

import os
import sys

import numpy as np

sys.path.insert(0, "/opt/trn_rl_repo")

import ml_dtypes  # noqa: E402

import concourse.bass as bass  # noqa: E402
import concourse.tile as tile  # noqa: E402
from concourse import mybir  # noqa: E402
from concourse.bass_utils import run_bass_kernel_spmd  # noqa: E402

B, T, N, C, K = 8, 32, 2048, 64, 32
NCORES = 8
TOK = (B * T) // NCORES     # 32
NBATCH = TOK // 4           # 8
NPAIR = TOK // 2            # 16
NCH = N // 128              # 16

BF16 = mybir.dt.bfloat16
F32 = mybir.dt.float32

_CACHE = {}

_NO_SPLIT_TYPES = ("InstEventSemaphore",)


def _split_excess_waits(nc):
    """walrus' setupSyncWait allows a single semaphore wait on several
    instruction structs; hoist extras onto standalone InstEventSemaphore."""
    for f in nc.m.functions:
        for blk in f.blocks:
            out = []
            changed = False
            for inst in blk.instructions:
                si = getattr(inst, "sync_info", None)
                if (
                    si is not None
                    and si.on_wait
                    and len(si.on_wait) > 1
                    and type(inst).__name__ not in _NO_SPLIT_TYPES
                ):
                    for idx, w in enumerate(si.on_wait[:-1]):
                        out.append(
                            mybir.InstEventSemaphore(
                                name=f"{inst.name}_xw{idx}",
                                engine=inst.engine,
                                sync_info=mybir.SyncInfo(on_wait=[w], on_update=[]),
                            )
                        )
                    inst.sync_info = mybir.SyncInfo(
                        on_wait=[si.on_wait[-1]], on_update=si.on_update
                    )
                    changed = True
                out.append(inst)
            if changed:
                try:
                    blk.instructions[:] = out
                except TypeError:
                    blk.instructions = out
    return nc


def _build_nc():
    nc = bass.Bass()
    RT = nc.declare_dram_parameter("RT", [NPAIR, 128, N], BF16, False)
    RN = nc.declare_dram_parameter("RN", [NBATCH, 128, NCH, 4, C + 1], BF16, False)
    W2 = nc.declare_dram_parameter("W2", [128, K], BF16, False)
    B128 = nc.declare_dram_parameter("B128", [128, 1], F32, False)
    IDN = nc.declare_dram_parameter("IDN", [128, 128], BF16, False)
    C4 = nc.declare_dram_parameter("C4", [128, C], F32, False)
    V = nc.declare_dram_parameter("V", [NBATCH, 128, C], F32, True)

    with tile.TileContext(nc) as tc:
        with (
            tc.tile_pool(name="singles", bufs=1) as singles,
            tc.tile_pool(name="rt", bufs=6) as rt_pool,
            tc.tile_pool(name="rn", bufs=3) as rn_pool,
            tc.tile_pool(name="e", bufs=5) as e_pool,
            tc.tile_pool(name="a", bufs=2) as a_pool,
            tc.tile_pool(name="s", bufs=2) as s_pool,
            tc.tile_pool(name="rs", bufs=3) as rs_pool,
            tc.tile_pool(name="o", bufs=2) as o_pool,
            tc.tile_pool(name="pq", bufs=2, space="PSUM") as pq_pool,
            tc.tile_pool(name="pet", bufs=3, space="PSUM") as pet_pool,
            tc.tile_pool(name="pv", bufs=1, space="PSUM") as pv_pool,
        ):
            w_sb = singles.tile([128, K], BF16)
            nc.sync.dma_start(out=w_sb[:], in_=W2[:])
            b_sb = singles.tile([128, 1], F32)
            nc.sync.dma_start(out=b_sb[:], in_=B128[:])
            idn_sb = singles.tile([128, 128], BF16)
            nc.sync.dma_start(out=idn_sb[:], in_=IDN[:])
            c4_sb = singles.tile([128, C], F32)
            nc.sync.dma_start(out=c4_sb[:], in_=C4[:])
            # force the Exp ACT table load during the DMA-fill window instead
            # of right before the first real exp
            warm = singles.tile([128, 1], F32)
            nc.vector.memset(warm[:], 0.0)
            nc.scalar.activation(
                warm[:], warm[:], mybir.ActivationFunctionType.Exp
            )

            rt_sb = [None] * NPAIR
            rn_sb = [None] * NBATCH
            a_sb = [None] * NBATCH   # per batch: 4 quarter tiles [128, 4, 4, K]
            pv = [None] * NBATCH

            def load_pair(p, split=False):
                rt_sb[p] = rt_pool.tile([128, N], BF16, name="rt_t", tag="rt_t")
                if split:
                    # two half-loads so the first G1 quarters start sooner
                    nc.sync.dma_start(
                        out=rt_sb[p][:, 0 : N // 2], in_=RT[p][:, 0 : N // 2]
                    )
                    nc.sync.dma_start(
                        out=rt_sb[p][:, N // 2 : N], in_=RT[p][:, N // 2 : N]
                    )
                else:
                    nc.sync.dma_start(out=rt_sb[p][:], in_=RT[p])

            def load_rn(bt):
                rn_sb[bt] = rn_pool.tile(
                    [128, NCH, 4, C + 1], BF16, name="rn_t", tag="rn_t"
                )
                nc.sync.dma_start(out=rn_sb[bt][:], in_=RN[bt])

            def g1(bt):
                """16 matmuls -> two [128, 2, 512] psum tiles; exp -> e quarters."""
                e_q = []
                for Qh in range(2):
                    ps2 = pq_pool.tile([128, 2, 512], F32, name="ps_t", tag="ps_t")
                    for j in range(4):
                        pr = 2 * bt + j // 2
                        q = j % 2
                        for Qq in range(2):
                            Q = 2 * Qh + Qq
                            nc.tensor.matmul(
                                ps2[32 * j : 32 * j + 32, Qq, :],
                                w_sb[64 * q : 64 * q + 64, :],
                                rt_sb[pr][64 * q : 64 * q + 64, 512 * Q : 512 * Q + 512],
                                start=True,
                                stop=True,
                                skip_group_check=True,
                                tile_position=(64 * q, 32 * j),
                            )
                    for Qq in range(2):
                        e = e_pool.tile([128, 512], BF16, name="e_t", tag="e_t")
                        nc.scalar.activation(
                            e[:],
                            ps2[:, Qq, :],
                            mybir.ActivationFunctionType.Exp,
                            bias=b_sb[:],
                        )
                        e_q.append(e)
                return e_q

            def softmax_quarter(bt, Q, e, s_b):
                """Transpose e-quarter (PE), reduce over k, 1/s, normalize."""
                et = pet_pool.tile([128, 4, 4, K], BF16, name="et_t", tag="et_t")
                for cq in range(4):
                    nc.tensor.transpose(
                        et[:, cq, :, :],
                        e[:, 128 * cq : 128 * cq + 128],
                        idn_sb[:],
                    )
                nc.vector.tensor_reduce(
                    s_b[:, Q, :, :],
                    et[:],
                    axis=mybir.AxisListType.X,
                    op=mybir.AluOpType.add,
                )
                rs_q = rs_pool.tile([128, 4, 4], F32, name="rs_t", tag="rs_t")
                nc.vector.reciprocal(rs_q[:], s_b[:, Q, :, :])
                aq = a_pool.tile([128, 4, 4, K], BF16, name=f"a{Q}_t", tag=f"a{Q}_t")
                nc.vector.tensor_mul(
                    aq[:], et[:], rs_q[:].unsqueeze(3).broadcast_to((128, 4, 4, K))
                )
                return aq

            def g2_quarter(bt, Q):
                """16 GEMM2 matmuls for chunks 4Q..4Q+3 (accumulating into pv)."""
                if Q == 0:
                    pv[bt] = pv_pool.tile([128, C + 1], F32, name="pv_t", tag="pv_t")
                pvt = pv[bt]
                rn = rn_sb[bt]
                aq = a_sb[bt][Q]
                for cq in range(4):
                    ch = 4 * Q + cq
                    for j in range(4):
                        nc.tensor.matmul(
                            pvt[32 * j : 32 * j + 32, :],
                            aq[:, cq, j, :],
                            rn[:, ch, j, :],
                            start=(ch == 0),
                            stop=(ch == NCH - 1),
                            skip_group_check=True,
                            tile_position=(0, 32 * j),
                        )

            def epilogue(bt):
                o = o_pool.tile([128, C], F32, name="o_t", tag="o_t")
                nc.vector.scalar_tensor_tensor(
                    o[:],
                    c4_sb[:],
                    pv[bt][:, C : C + 1],
                    pv[bt][:, 0:C],
                    op0=mybir.AluOpType.mult,
                    op1=mybir.AluOpType.add,
                )
                nc.sync.dma_start(out=V[bt], in_=o[:])
                pv[bt] = None
                rn_sb[bt] = None
                a_sb[bt] = None

            load_pair(0, split=True)
            load_pair(1, split=True)
            for bt in range(NBATCH + 1):
                if bt < NBATCH:
                    if 2 * bt + 3 < NPAIR:
                        load_pair(2 * bt + 2)
                        load_pair(2 * bt + 3)
                    load_rn(bt)
                    e_q = g1(bt)
                    # interleave transpose bursts (HAM-idle for the PE) with
                    # GEMM2 matmul groups of the previous batch to keep the
                    # PE clock gate warm
                    s_b = s_pool.tile([128, 4, 4, 4], F32, name="s_t", tag="s_t")
                    quarters = []
                    for Q in range(4):
                        quarters.append(softmax_quarter(bt, Q, e_q[Q], s_b))
                        if bt >= 1:
                            g2_quarter(bt - 1, Q)
                    a_sb[bt] = quarters
                    rt_sb[2 * bt] = None
                    rt_sb[2 * bt + 1] = None
                else:
                    for Q in range(4):
                        g2_quarter(bt - 1, Q)
                if bt >= 1:
                    epilogue(bt - 1)
    return _split_excess_waits(nc)


def kernel(R_seq, W, b, centroids):
    if "nc" not in _CACHE:
        _CACHE["nc"] = _build_nc()
    nc = _CACHE["nc"]
    bf = ml_dtypes.bfloat16

    WT = np.ascontiguousarray(W.astype(np.float32).T)              # [C, K]
    W2_h = np.concatenate([WT, WT], axis=0).astype(bf)             # [128, K]
    B128_h = np.ascontiguousarray(
        np.tile(b.astype(np.float32), 4).reshape(128, 1)
    )
    IDN_h = np.eye(128, dtype=np.float32).astype(bf)
    C4_h = np.ascontiguousarray(np.tile(centroids.astype(np.float32), (4, 1)))

    r_all = R_seq.astype(np.float32).reshape(NCORES, TOK, N, C)
    in_maps = []
    for i in range(NCORES):
        rc = r_all[i]
        r6 = rc.reshape(NPAIR, 2, N, C).transpose(0, 1, 3, 2)      # [p, t, c, n]
        RT_h = np.ascontiguousarray(r6).reshape(NPAIR, 128, N).astype(bf)
        r7 = rc.reshape(NBATCH, 4, NCH, 128, C).transpose(0, 3, 2, 1, 4)
        RN_h = np.concatenate(
            [r7, np.full(r7.shape[:-1] + (1,), -1.0, np.float32)], axis=-1
        ).astype(bf)                                               # [bt,128,ch,j,65]
        in_maps.append(
            {
                "RT": RT_h,
                "RN": np.ascontiguousarray(RN_h),
                "W2": W2_h,
                "B128": B128_h,
                "IDN": IDN_h,
                "C4": C4_h,
            }
        )

    res = run_bass_kernel_spmd(
        nc,
        in_maps,
        list(range(NCORES)),
        trace=bool(int(os.environ.get("NETVLAD_TRACE", "0"))),
    )
    _CACHE["last_results"] = res

    outs = []
    for i in range(NCORES):
        v = np.asarray(res.results[i]["V"], np.float32)   # [NBATCH, 128, C]
        outs.append(v.reshape(TOK, K, C))
    return np.stack(outs, axis=0).reshape(B, T, K, C).astype(np.float32)


if __name__ == "__main__":
    rng = np.random.default_rng(0)
    R = rng.normal(size=(B, T, N, C)).astype(np.float32)
    W_ = rng.normal(size=(K, C)).astype(np.float32) / 8.0
    b_ = (rng.normal(size=(K,)) * 0.01).astype(np.float32)
    cc = rng.normal(size=(K, C)).astype(np.float32)
    out = kernel(R, W_, b_, cc)
    print(out.shape, out.dtype)


# revision 6
# speedup vs baseline: 1.4079x; 1.0617x over previous
"""NetVLAD pooling kernel v2 for Trainium2 (8 NeuronCores, data-parallel over B).

Per core: 32 tokens, r = [N=2048, C=64] each; logits = r @ W.T + b;
a = softmax(logits); v = a.T @ r - sum(a).T * centroids   -> [K=32, C=64].

Design (vs the two-GEMM baseline, measured on HW):
  - GEMM1 runs "transposed" with W STATIONARY: the baseline made each rT chunk
    the stationary operand, paying a 128-col LDWEIGHTS (~107ns) per chunk
    (~55us/core of PE weight-load). Here 4 copies of W.T sit in the PE at
    tile_position (64*(j%2), 32*j) (row-pair x 4 col-strips, one per token of
    a 4-token batch) and r streams as the MOVING operand in N=512 quarters.
    Issuing the 2 same-weight quarters back-to-back pairs the streams in the
    array: measured ~108ns per N=512 matmul -> GEMM1 ~14us/core.
    Output: logitsT psum [128 = 4 tok x 32 k, 512 n].
  - b folds into the exp via ACTIVATE's per-partition bias (k is the partition
    dim here), deleting the baseline's beta-multiply pass on VectorE.
  - a-tiles for GEMM2 need [n, k] layout: PE transpose-mode on the exp'd bf16
    tiles ([128,128] blocks, ~108-200ns each, output psum-bf16; measured exact)
    -> [n-part, (cq, tok, k)]. Softmax denominator = VectorE segmented reduce
    on the psum-bf16 tile; 1/s on GpSimd (ALU divide, bf16 out); the normalize
    multiply doubles as the psum->SBUF evacuation (all-bf16 for DVE rate).
  - GEMM2 = baseline scheme: per (chunk, token) a-tile [128, 32] stationary,
    rhs = RN [128 n, 65] with a trailing -1 column so psum col 64 = -sum(a);
    4 tokens col-tiled into one psum bank; measured ~33ns/MM -> ~17us/core.
  - Epilogue: v = c4 * pv[:, 64] + pv[:, :64] (one scalar_tensor_tensor).
  - GEMM2 lags the softmax chain by one batch; loads prefetch one batch ahead.
"""

import os
import sys

import numpy as np

sys.path.insert(0, "/opt/trn_rl_repo")

import ml_dtypes  # noqa: E402

import concourse.bass as bass  # noqa: E402
import concourse.tile as tile  # noqa: E402
from concourse import mybir  # noqa: E402
from concourse.bass_utils import run_bass_kernel_spmd  # noqa: E402

B, T, N, C, K = 8, 32, 2048, 64, 32
NCORES = 8
TOK = (B * T) // NCORES     # 32
NBATCH = TOK // 4           # 8
NPAIR = TOK // 2            # 16
NCH = N // 128              # 16

BF16 = mybir.dt.bfloat16
F32 = mybir.dt.float32

_CACHE = {}

_NO_SPLIT_TYPES = ("InstEventSemaphore",)


def _split_excess_waits(nc):
    """walrus' setupSyncWait allows a single semaphore wait on several
    instruction structs; hoist extras onto standalone InstEventSemaphore."""
    for f in nc.m.functions:
        for blk in f.blocks:
            out = []
            changed = False
            for inst in blk.instructions:
                si = getattr(inst, "sync_info", None)
                if (
                    si is not None
                    and si.on_wait
                    and len(si.on_wait) > 1
                    and type(inst).__name__ not in _NO_SPLIT_TYPES
                ):
                    for idx, w in enumerate(si.on_wait[:-1]):
                        out.append(
                            mybir.InstEventSemaphore(
                                name=f"{inst.name}_xw{idx}",
                                engine=inst.engine,
                                sync_info=mybir.SyncInfo(on_wait=[w], on_update=[]),
                            )
                        )
                    inst.sync_info = mybir.SyncInfo(
                        on_wait=[si.on_wait[-1]], on_update=si.on_update
                    )
                    changed = True
                out.append(inst)
            if changed:
                try:
                    blk.instructions[:] = out
                except TypeError:
                    blk.instructions = out
    return nc


def _build_nc():
    nc = bass.Bass()
    RT = nc.declare_dram_parameter("RT", [NPAIR, 128, N], BF16, False)
    RN = nc.declare_dram_parameter("RN", [NBATCH, 128, NCH, 4, C + 1], BF16, False)
    W2 = nc.declare_dram_parameter("W2", [128, K], BF16, False)
    B128 = nc.declare_dram_parameter("B128", [128, 1], F32, False)
    IDN = nc.declare_dram_parameter("IDN", [128, 128], BF16, False)
    C4 = nc.declare_dram_parameter("C4", [128, C], F32, False)
    V = nc.declare_dram_parameter("V", [NBATCH, 128, C], F32, True)

    with tile.TileContext(nc) as tc:
        with (
            tc.tile_pool(name="singles", bufs=1) as singles,
            tc.tile_pool(name="rt", bufs=6) as rt_pool,
            tc.tile_pool(name="rn", bufs=3) as rn_pool,
            tc.tile_pool(name="e", bufs=5) as e_pool,
            tc.tile_pool(name="a", bufs=2) as a_pool,
            tc.tile_pool(name="s", bufs=2) as s_pool,
            tc.tile_pool(name="rs", bufs=3) as rs_pool,
            tc.tile_pool(name="o", bufs=2) as o_pool,
            tc.tile_pool(name="pq", bufs=2, space="PSUM") as pq_pool,
            tc.tile_pool(name="pet", bufs=3, space="PSUM") as pet_pool,
            tc.tile_pool(name="pv", bufs=1, space="PSUM") as pv_pool,
        ):
            w_sb = singles.tile([128, K], BF16)
            nc.sync.dma_start(out=w_sb[:], in_=W2[:])
            b_sb = singles.tile([128, 1], F32)
            nc.sync.dma_start(out=b_sb[:], in_=B128[:])
            idn_sb = singles.tile([128, 128], BF16)
            nc.sync.dma_start(out=idn_sb[:], in_=IDN[:])
            c4_sb = singles.tile([128, C], F32)
            nc.sync.dma_start(out=c4_sb[:], in_=C4[:])
            # force the Exp ACT table load during the DMA-fill window instead
            # of right before the first real exp
            warm = singles.tile([128, 1], F32)
            nc.vector.memset(warm[:], 0.0)
            nc.scalar.activation(
                warm[:], warm[:], mybir.ActivationFunctionType.Exp
            )

            rt_sb = [None] * NPAIR
            rn_sb = [None] * NBATCH
            a_sb = [None] * NBATCH   # per batch: 4 quarter tiles [128, 4, 4, K]
            pv = [None] * NBATCH

            def load_pair(p, split=False):
                rt_sb[p] = rt_pool.tile([128, N], BF16, name="rt_t", tag="rt_t")
                if split:
                    # two half-loads so the first G1 quarters start sooner
                    nc.sync.dma_start(
                        out=rt_sb[p][:, 0 : N // 2], in_=RT[p][:, 0 : N // 2]
                    )
                    nc.sync.dma_start(
                        out=rt_sb[p][:, N // 2 : N], in_=RT[p][:, N // 2 : N]
                    )
                else:
                    nc.sync.dma_start(out=rt_sb[p][:], in_=RT[p])

            def load_rn(bt):
                rn_sb[bt] = rn_pool.tile(
                    [128, NCH, 4, C + 1], BF16, name="rn_t", tag="rn_t"
                )
                nc.sync.dma_start(out=rn_sb[bt][:], in_=RN[bt])

            def g1(bt):
                """16 matmuls -> two [128, 2, 512] psum tiles; exp -> e quarters."""
                e_q = []
                for Qh in range(2):
                    ps2 = pq_pool.tile([128, 2, 512], F32, name="ps_t", tag="ps_t")
                    for j in range(4):
                        pr = 2 * bt + j // 2
                        q = j % 2
                        for Qq in range(2):
                            Q = 2 * Qh + Qq
                            nc.tensor.matmul(
                                ps2[32 * j : 32 * j + 32, Qq, :],
                                w_sb[64 * q : 64 * q + 64, :],
                                rt_sb[pr][64 * q : 64 * q + 64, 512 * Q : 512 * Q + 512],
                                start=True,
                                stop=True,
                                skip_group_check=True,
                                tile_position=(64 * q, 32 * j),
                            )
                    for Qq in range(2):
                        e = e_pool.tile([128, 512], BF16, name="e_t", tag="e_t")
                        nc.scalar.activation(
                            e[:],
                            ps2[:, Qq, :],
                            mybir.ActivationFunctionType.Exp,
                            bias=b_sb[:],
                        )
                        e_q.append(e)
                return e_q

            def softmax_quarter(bt, Q, e, s_b):
                """Transpose e-quarter (PE), reduce over k, 1/s, normalize."""
                et = pet_pool.tile([128, 4, 4, K], BF16, name="et_t", tag="et_t")
                for cq in range(4):
                    nc.tensor.transpose(
                        et[:, cq, :, :],
                        e[:, 128 * cq : 128 * cq + 128],
                        idn_sb[:],
                    )
                nc.vector.tensor_reduce(
                    s_b[:, Q, :, :],
                    et[:],
                    axis=mybir.AxisListType.X,
                    op=mybir.AluOpType.add,
                )
                rs_q = rs_pool.tile([128, 4, 4], F32, name="rs_t", tag="rs_t")
                nc.vector.reciprocal(rs_q[:], s_b[:, Q, :, :])
                aq = a_pool.tile([128, 4, 4, K], BF16, name=f"a{Q}_t", tag=f"a{Q}_t")
                nc.vector.tensor_mul(
                    aq[:], et[:], rs_q[:].unsqueeze(3).broadcast_to((128, 4, 4, K))
                )
                return aq

            def g2_quarter(bt, Q):
                """16 GEMM2 matmuls for chunks 4Q..4Q+3 (accumulating into pv)."""
                if Q == 0:
                    pv[bt] = pv_pool.tile([128, C + 1], F32, name="pv_t", tag="pv_t")
                pvt = pv[bt]
                rn = rn_sb[bt]
                aq = a_sb[bt][Q]
                for cq in range(4):
                    ch = 4 * Q + cq
                    for j in range(4):
                        nc.tensor.matmul(
                            pvt[32 * j : 32 * j + 32, :],
                            aq[:, cq, j, :],
                            rn[:, ch, j, :],
                            start=(ch == 0),
                            stop=(ch == NCH - 1),
                            skip_group_check=True,
                            tile_position=(0, 32 * j),
                        )

            def epilogue(bt):
                o = o_pool.tile([128, C], F32, name="o_t", tag="o_t")
                nc.vector.scalar_tensor_tensor(
                    o[:],
                    c4_sb[:],
                    pv[bt][:, C : C + 1],
                    pv[bt][:, 0:C],
                    op0=mybir.AluOpType.mult,
                    op1=mybir.AluOpType.add,
                )
                nc.sync.dma_start(out=V[bt], in_=o[:])
                pv[bt] = None
                rn_sb[bt] = None
                a_sb[bt] = None

            load_pair(0, split=True)
            load_pair(1, split=True)

            # Software-pipelined softmax across global quarters gq = 4*bt + Q:
            # each step issues [transposes+reduce](gq), recip(gq-1),
            # norm(gq-2).  The reduce->recip->norm chain is spread over three
            # steps so every VectorE op reaches its in-order queue head with
            # its dependency already complete (no ~600ns completion-semaphore
            # head-blocking per link).  et psum tiles live exactly 3 steps
            # (= pet bufs).
            NQ = NBATCH * 4
            stq = {}   # gq -> et psum tile (until its norm)
            rsq = {}   # gq -> rs tile (until its norm)
            sbt = {}   # bt -> s_b tile
            e_cur = [None]

            def stage_tr(gq):
                bt, Q = divmod(gq, 4)
                et = pet_pool.tile([128, 4, 4, K], BF16, name="et_t", tag="et_t")
                for cq in range(4):
                    nc.tensor.transpose(
                        et[:, cq, :, :],
                        e_cur[0][Q][:, 128 * cq : 128 * cq + 128],
                        idn_sb[:],
                    )
                nc.vector.tensor_reduce(
                    sbt[bt][:, Q, :, :],
                    et[:],
                    axis=mybir.AxisListType.X,
                    op=mybir.AluOpType.add,
                )
                stq[gq] = et

            def stage_recip(gq):
                bt, Q = divmod(gq, 4)
                rs = rs_pool.tile([128, 4, 4], F32, name="rs_t", tag="rs_t")
                nc.vector.reciprocal(rs[:], sbt[bt][:, Q, :, :])
                rsq[gq] = rs

            def stage_norm(gq):
                bt, Q = divmod(gq, 4)
                aq = a_pool.tile([128, 4, 4, K], BF16, name=f"a{Q}_t", tag=f"a{Q}_t")
                nc.vector.tensor_mul(
                    aq[:],
                    stq[gq][:],
                    rsq[gq][:].unsqueeze(3).broadcast_to((128, 4, 4, K)),
                )
                stq[gq] = None
                rsq[gq] = None
                a_sb[bt].append(aq)

            for gq in range(NQ):
                bt, Q = divmod(gq, 4)
                if Q == 0:
                    if 2 * bt + 3 < NPAIR:
                        load_pair(2 * bt + 2)
                        load_pair(2 * bt + 3)
                    load_rn(bt)
                    e_cur[0] = g1(bt)
                    sbt[bt] = s_pool.tile(
                        [128, 4, 4, 4], F32, name="s_t", tag="s_t"
                    )
                    a_sb[bt] = []
                stage_tr(gq)
                if bt >= 1:
                    g2_quarter(bt - 1, Q)
                if gq >= 1:
                    stage_recip(gq - 1)
                if gq >= 2:
                    stage_norm(gq - 2)
                if Q == 3:
                    rt_sb[2 * bt] = None
                    rt_sb[2 * bt + 1] = None
                    if bt >= 1:
                        epilogue(bt - 1)
            # flush the pipeline and the final batch's GEMM2
            stage_recip(NQ - 1)
            stage_norm(NQ - 2)
            stage_norm(NQ - 1)
            for Q in range(4):
                g2_quarter(NBATCH - 1, Q)
            epilogue(NBATCH - 1)
    return _split_excess_waits(nc)


def kernel(R_seq, W, b, centroids):
    if "nc" not in _CACHE:
        _CACHE["nc"] = _build_nc()
    nc = _CACHE["nc"]
    bf = ml_dtypes.bfloat16

    WT = np.ascontiguousarray(W.astype(np.float32).T)              # [C, K]
    W2_h = np.concatenate([WT, WT], axis=0).astype(bf)             # [128, K]
    B128_h = np.ascontiguousarray(
        np.tile(b.astype(np.float32), 4).reshape(128, 1)
    )
    IDN_h = np.eye(128, dtype=np.float32).astype(bf)
    C4_h = np.ascontiguousarray(np.tile(centroids.astype(np.float32), (4, 1)))

    r_all = R_seq.astype(np.float32).reshape(NCORES, TOK, N, C)
    in_maps = []
    for i in range(NCORES):
        rc = r_all[i]
        r6 = rc.reshape(NPAIR, 2, N, C).transpose(0, 1, 3, 2)      # [p, t, c, n]
        RT_h = np.ascontiguousarray(r6).reshape(NPAIR, 128, N).astype(bf)
        r7 = rc.reshape(NBATCH, 4, NCH, 128, C).transpose(0, 3, 2, 1, 4)
        RN_h = np.concatenate(
            [r7, np.full(r7.shape[:-1] + (1,), -1.0, np.float32)], axis=-1
        ).astype(bf)                                               # [bt,128,ch,j,65]
        in_maps.append(
            {
                "RT": RT_h,
                "RN": np.ascontiguousarray(RN_h),
                "W2": W2_h,
                "B128": B128_h,
                "IDN": IDN_h,
                "C4": C4_h,
            }
        )

    res = run_bass_kernel_spmd(
        nc,
        in_maps,
        list(range(NCORES)),
        trace=bool(int(os.environ.get("NETVLAD_TRACE", "0"))),
    )
    _CACHE["last_results"] = res

    outs = []
    for i in range(NCORES):
        v = np.asarray(res.results[i]["V"], np.float32)   # [NBATCH, 128, C]
        outs.append(v.reshape(TOK, K, C))
    return np.stack(outs, axis=0).reshape(B, T, K, C).astype(np.float32)


if __name__ == "__main__":
    rng = np.random.default_rng(0)
    R = rng.normal(size=(B, T, N, C)).astype(np.float32)
    W_ = rng.normal(size=(K, C)).astype(np.float32) / 8.0
    b_ = (rng.normal(size=(K,)) * 0.01).astype(np.float32)
    cc = rng.normal(size=(K, C)).astype(np.float32)
    out = kernel(R, W_, b_, cc)
    print(out.shape, out.dtype)
